# revision 12
# baseline (speedup 1.0000x reference)
"""Multi-head attention (B=4, S=2048, D=1024, H=16) on 8 Trainium2 cores.

Sharding: data parallel on batch (4) x tensor parallel on heads (2 halves of
8 heads). Core c handles batch c//2 and head-half c%2: column-parallel
w_q/w_k/w_v (512 out dims), local attention over its 8 heads, row-parallel
w_o (its 512 hd columns) producing a full [2048, 1024] partial that the host
sums across the two halves (plus b_o).

On-device layout is feature-on-partitions throughout ("transposed"):
  qP/kP: [dout 512 -> 4 ptiles, seq 2048] bf16   (projection form B)
  scores S.T: [keys, queries] via paired K=64 matmuls (head pair at PE row
  offsets 0/64 with tile_position) into a 2-bank PSUM tile, one wide exp ACT
  AV: O.T accumulation with V_aug ones-column producing row sums; normalize
  via DVE fast reciprocal + GpSimd partition-broadcast; out-proj form A from
  attnT [hd, seq] giving the natural [seq, dout] partial.

The attention phase is ScalarE(exp)-bound; all projection and out-projection
matmuls are drip-fed into the PE queue between attention steps to keep the
PE dense (HAM stays at K=8/8) without starving the ACT pipeline.
"""

import time
from collections import deque
from contextlib import ExitStack

import ml_dtypes
import numpy as np

import concourse.bass as bass
import concourse.mybir as mybir
import concourse.tile as tile
from concourse import bacc
from concourse.bass import ds, ts
from concourse.bass_utils import run_bass_kernel_spmd

F32 = mybir.dt.float32
BF16 = mybir.dt.bfloat16
EXP = mybir.ActivationFunctionType.Exp
MULT = mybir.AluOpType.mult
BF = ml_dtypes.bfloat16

B, S, D, H, DH = 4, 2048, 1024, 16, 64
HALF = D // 2          # 512 douts per core
DT = HALF // 128       # 4 dout tiles
DIN = D // 128         # 8 din tiles
QB = S // 512          # 4 query blocks
KT = S // 128          # 16 key tiles / seq tiles

TRACE = False
USE_POOL_BCAST = True
DEBUG_DUMP = False
LAST_EXEC_NS = None
LAST_TRACE = None
_NC = None


def _build(DEBUG_DUMP=False):
    nc = bacc.Bacc("TRN2", target_bir_lowering=False, debug=False,
                   num_devices=8, name="mha")

    qT_d = nc.dram_tensor("qT", [D, S], BF16, kind="ExternalInput")
    kT_d = nc.dram_tensor("kT", [D, S], BF16, kind="ExternalInput")
    vT_d = nc.dram_tensor("vT", [D, S], BF16, kind="ExternalInput")
    wq_d = nc.dram_tensor("wq", [D, HALF], BF16, kind="ExternalInput")
    wk_d = nc.dram_tensor("wk", [D, HALF], BF16, kind="ExternalInput")
    wv_d = nc.dram_tensor("wv", [D, HALF], BF16, kind="ExternalInput")
    wo_d = nc.dram_tensor("wo", [HALF, D], BF16, kind="ExternalInput")
    bq_d = nc.dram_tensor("bq", [1, HALF], BF16, kind="ExternalInput")
    bk_d = nc.dram_tensor("bk", [1, HALF], BF16, kind="ExternalInput")
    bv_d = nc.dram_tensor("bv", [1, HALF], BF16, kind="ExternalInput")
    out_d = nc.dram_tensor("out", [S, D], F32, kind="ExternalOutput")
    if DEBUG_DUMP:
        dbg_qP = nc.dram_tensor("dbg_qP", [128, DT, S], BF16, kind="ExternalOutput")
        dbg_kP = nc.dram_tensor("dbg_kP", [128, DT, S], BF16, kind="ExternalOutput")
        dbg_va = nc.dram_tensor("dbg_va", [128, KT, 8 * 65], BF16, kind="ExternalOutput")
        dbg_at = nc.dram_tensor("dbg_at", [128, DT, S], BF16, kind="ExternalOutput")
        dbg_oa = nc.dram_tensor("dbg_oa", [16, 128, 1024], F32, kind="ExternalOutput")

    stk = ExitStack()
    with tile.TileContext(nc) as tc:
        persist = stk.enter_context(tc.tile_pool(name="persist", bufs=1))
        xin = stk.enter_context(tc.tile_pool(name="xin", bufs=16))
        qch = stk.enter_context(tc.tile_pool(name="qch", bufs=12))
        pTp = stk.enter_context(tc.tile_pool(name="pTp", bufs=3))
        otsb = stk.enter_context(tc.tile_pool(name="otsb", bufs=3))
        nrm = stk.enter_context(tc.tile_pool(name="nrm", bufs=1))
        outsb = stk.enter_context(tc.tile_pool(name="outsb", bufs=2))
        ps_pair = stk.enter_context(tc.tile_pool(name="ps_pair", bufs=2, space="PSUM"))
        ps_ot = stk.enter_context(tc.tile_pool(name="ps_ot", bufs=2, space="PSUM"))
        ps_proj = stk.enter_context(tc.tile_pool(name="ps_proj", bufs=2, space="PSUM"))

        # --- persistent SBUF ---
        wq_sb = persist.tile([128, DIN, HALF], BF16)
        wk_sb = persist.tile([128, DIN, HALF], BF16)
        wv_sb = persist.tile([128, DIN, HALF], BF16)
        wo_sb = persist.tile([128, DT, D], BF16)
        bq_sb = persist.tile([1, HALF], BF16)
        bk_sb = persist.tile([1, HALF], BF16)
        bv_sb = persist.tile([1, HALF], BF16)
        ones_row = persist.tile([1, S], BF16)
        ones_col = persist.tile([1, 64], F32)
        nc.vector.memset(ones_col[:], 1.0)
        qP = persist.tile([128, DT, S], BF16)
        kP = persist.tile([128, DT, S], BF16)
        v_aug = persist.tile([128, KT, 8 * 65], BF16)
        attnT = persist.tile([128, DT, S], BF16)

        nc.sync.dma_start(wk_sb[:], wk_d[:].rearrange("(o p) n -> p o n", p=128))
        nc.sync.dma_start(bk_sb[:], bk_d[:])
        nc.vector.memset(ones_row[:], 1.0)
        nc.vector.memset(v_aug[:], 1.0)

        kin = []
        for d in range(DIN):
            t = xin.tile([128, S], BF16, tag="xin")
            nc.sync.dma_start(t[:], kT_d[:].rearrange("(o p) f -> o p f", p=128)[d])
            kin.append(t)
        nc.sync.dma_start(wv_sb[:], wv_d[:].rearrange("(o p) n -> p o n", p=128))
        nc.sync.dma_start(bv_sb[:], bv_d[:])
        vin = []
        for d in range(DIN):
            t = xin.tile([128, S], BF16, tag="xin")
            nc.sync.dma_start(t[:], vT_d[:].rearrange("(o p) f -> o p f", p=128)[d])
            vin.append(t)
        nc.sync.dma_start(wq_sb[:], wq_d[:].rearrange("(o p) n -> p o n", p=128))
        nc.sync.dma_start(bq_sb[:], bq_d[:])
        nc.sync.dma_start(wo_sb[:], wo_d[:].rearrange("(o p) n -> p o n", p=128))

        # q input is loaded in [128, 512] chunks per (din, qb) to cap residency
        qchunks = {}

        def load_qchunks(qb):
            for d in range(DIN):
                t = qch.tile([128, 512], BF16, tag="qch")
                nc.sync.dma_start(
                    t[:], qT_d[:].rearrange("(o p) f -> o p f", p=128)[d][:, ts(qb, 512)])
                qchunks[(d, qb)] = t

        def qk_proj(src, w_sb, b_sb, oP, dt, qb):
            """Emit one [dout-tile, 512-queries] projection group (9 MMs + copy).

            src: callable d -> [128, 512] bf16 AP for that din tile."""
            ps = ps_proj.tile([128, 512], F32, tag="proj")
            for d in range(DIN):
                nc.tensor.matmul(ps[:], w_sb[:, d, ts(dt, 128)], src(d),
                                 start=(d == 0), stop=False)
            nc.tensor.matmul(ps[:], b_sb[0:1, ts(dt, 128)],
                             ones_row[0:1, ts(qb, 512)], start=False, stop=True)
            nc.vector.tensor_copy(oP[:, dt, ts(qb, 512)], ps[:])

        def qk_proj_items(src, w_sb, b_sb, oP, dt, qb):
            """qk_proj split into 2-MM drip-feedable emission closures, fine
            enough to fit the per-step PE slack without delaying scores."""
            state = {}

            def mk_mm(d0):
                def mm():
                    if d0 == 0:
                        ps = ps_proj.tile([128, 512], F32, tag="proj",
                                          name="proj_ps")
                        state["ps"] = ps
                    ps = state["ps"]
                    for d in (d0, d0 + 1):
                        nc.tensor.matmul(ps[:], w_sb[:, d, ts(dt, 128)], src(d),
                                         start=(d == 0), stop=False)
                return mm

            def wb():
                ps = state["ps"]
                nc.tensor.matmul(ps[:], b_sb[0:1, ts(dt, 128)],
                                 ones_row[0:1, ts(qb, 512)], start=False, stop=True)
                nc.vector.tensor_copy(oP[:, dt, ts(qb, 512)], ps[:])

            return [mk_mm(0), mk_mm(2), mk_mm(4), mk_mm(6), wb]

        def kproj_items(dt, qbk):
            return qk_proj_items(lambda d, q=qbk: kin[d][:, ts(q, 512)],
                                 wk_sb, bk_sb, kP, dt, qbk)

        # ---- upfront ramp: Kproj dt0, Vproj st0..5, Qproj (dt0, qb0);
        # the rest drip-feeds into the attention blocks below ----
        for qbk in range(QB):
            qk_proj(lambda d, q=qbk: kin[d][:, ts(q, 512)], wk_sb, bk_sb,
                    kP, 0, qbk)
        load_qchunks(0)
        def v_proj(st):
            ps = ps_proj.tile([128, 512], F32, tag="proj")
            for d in range(DIN):
                nc.tensor.matmul(ps[:], vin[d][:, ts(st, 128)], wv_sb[:, d, :],
                                 start=(d == 0), stop=False)
            nc.tensor.matmul(ps[:], ones_row[0:1, ts(st, 128)], bv_sb[0:1, :],
                             start=False, stop=True)
            nc.vector.tensor_copy(
                v_aug[:, st].rearrange("p (h c) -> p h c", h=8)[:, :, 0:64],
                ps[:].rearrange("p (h c) -> p h c", h=8))

        for st in range(6):
            v_proj(st)
        qk_proj(lambda d: qchunks[(d, 0)][:], wq_sb, bq_sb, qP, 0, 0)

        def outproj_items(qb):
            """Out-projection for query block qb as a list of fine-grained
            emission closures (PE filler)."""
            items = []
            for j in range(4):
                st = qb * 4 + j
                for half in range(2):
                    state = {}

                    def mk(st=st, half=half, state=state):
                        def mm_a():
                            ps = ps_proj.tile([128, 512], F32, tag="proj")
                            state["ps"] = ps
                            for dt in (0, 1):
                                nc.tensor.matmul(ps[:], attnT[:, dt, ts(st, 128)],
                                                 wo_sb[:, dt, ts(half, 512)],
                                                 start=(dt == 0), stop=False)

                        def mm_b():
                            ps = state["ps"]
                            for dt in (2, 3):
                                nc.tensor.matmul(ps[:], attnT[:, dt, ts(st, 128)],
                                                 wo_sb[:, dt, ts(half, 512)],
                                                 start=False, stop=(dt == 3))

                        def wb():
                            ps = state["ps"]
                            osb = outsb.tile([128, 512], F32, tag="osb")
                            nc.vector.tensor_copy(osb[:], ps[:])
                            nc.sync.dma_start(
                                out_d[ds(st * 128, 128), ts(half, 512)], osb[:])

                        return [mm_a, mm_b, wb]

                    items += mk()
            return items

        def qproj_items(dt, qb):
            return qk_proj_items(lambda d, q=qb: qchunks[(d, q)][:],
                                 wq_sb, bq_sb, qP, dt, qb)

        # ---- attention: qb outer, head-pair inner, ACT-bound steady state ----
        for qb in range(QB):
            if qb < QB - 1:
                load_qchunks(qb + 1)
            for hp in range(DT):
                fillers = deque()
                if qb == 0:
                    if hp < 3:
                        for qbk in range(QB):
                            fillers.extend(kproj_items(hp + 1, qbk))
                        fillers.extend(qproj_items(hp + 1, 0))
                    else:
                        for dt in range(DT):
                            fillers.extend(qproj_items(dt, 1))
                else:
                    if qb < QB - 1:
                        fillers.extend(qproj_items(hp, qb + 1))
                    fillers.extend(outproj_items(qb - 1)[hp * 6:(hp + 1) * 6])

                otA = ps_ot.tile([128, 512], F32, tag="ot")
                otB = ps_ot.tile([128, 512], F32, tag="ot")
                prev_p = None
                for kt in range(KT):
                    if qb == 0 and hp == 0 and 6 + kt < KT:
                        v_proj(6 + kt)
                    pair = ps_pair.tile([128, 1024], F32, tag="pair")
                    nc.tensor.matmul(pair[:, 0:512],
                                     kP[0:64, hp, ts(kt, 128)],
                                     qP[0:64, hp, ts(qb, 512)],
                                     start=True, stop=True, tile_position=(0, 0))
                    nc.tensor.matmul(pair[:, 512:1024],
                                     kP[64:128, hp, ts(kt, 128)],
                                     qP[64:128, hp, ts(qb, 512)],
                                     start=True, stop=True, tile_position=(64, 0))
                    p = pTp.tile([128, 1024], BF16, tag="pT")
                    nc.scalar.activation(p[:], pair[:], EXP, scale=0.125)
                    # AV for the previous step: keeps scores one step ahead of
                    # the exp results in the PE queue (no PE wait on ACT)
                    if prev_p is not None:
                        pkt, pp = prev_p
                        nc.tensor.matmul(otA[0:65, :],
                                         v_aug[:, pkt, ds(2 * hp * 65, 65)],
                                         pp[:, 0:512],
                                         start=(pkt == 0), stop=False)
                        nc.tensor.matmul(otB[0:65, :],
                                         v_aug[:, pkt, ds((2 * hp + 1) * 65, 65)],
                                         pp[:, 512:1024],
                                         start=(pkt == 0), stop=False)
                    prev_p = (kt, p)
                    steps_left = KT - kt
                    pops = min(len(fillers), max(1, -(-len(fillers) // steps_left)))
                    for _ in range(pops):
                        fillers.popleft()()
                pkt, pp = prev_p
                nc.tensor.matmul(otA[0:65, :], v_aug[:, pkt, ds(2 * hp * 65, 65)],
                                 pp[:, 0:512], start=False, stop=True)
                nc.tensor.matmul(otB[0:65, :],
                                 v_aug[:, pkt, ds((2 * hp + 1) * 65, 65)],
                                 pp[:, 512:1024], start=False, stop=True)
                while fillers:
                    fillers.popleft()()

                # drain OT to SBUF (frees the PSUM banks), then normalize off
                # the PE-critical path
                oa = otsb.tile([128, 512], F32, tag="ot_sb")
                ob = otsb.tile([128, 512], F32, tag="ot_sb")
                nc.vector.tensor_copy(oa[0:64, :], otA[0:64, :])
                nc.vector.tensor_copy(ob[0:64, :], otB[0:64, :])
                # sums rows to partition 0: custom DVE ops require base
                # partition 0 operands
                sm = nrm.tile([1, 1024], F32, tag="sums")
                nc.vector.tensor_copy(sm[0:1, 0:512], otA[64:65, :])
                nc.vector.tensor_copy(sm[0:1, 512:1024], otB[64:65, :])
                if DEBUG_DUMP:
                    nc.sync.dma_start(dbg_oa[qb * 4 + hp, :, 0:512], oa[:])
                    nc.sync.dma_start(dbg_oa[qb * 4 + hp, :, 512:1024], ob[:])
                r = nrm.tile([1, 1024], F32, tag="recip")
                nc.vector.reciprocal_approx_fast(r[0:1, :], sm[0:1, :])
                rb = nrm.tile([64, 1024], F32, tag="rb")
                if USE_POOL_BCAST:
                    nc.gpsimd.partition_broadcast(rb[:], r[0:1, :])
                else:
                    rbp = ps_pair.tile([128, 1024], F32, tag="pair")
                    nc.tensor.matmul(rbp[0:64, 0:512], ones_col[0:1, :],
                                     r[0:1, 0:512], start=True, stop=True)
                    nc.tensor.matmul(rbp[0:64, 512:1024], ones_col[0:1, :],
                                     r[0:1, 512:1024], start=True, stop=True)
                    nc.vector.tensor_copy(rb[:], rbp[0:64, :])
                nc.vector.tensor_tensor(attnT[0:64, hp, ts(qb, 512)],
                                        oa[0:64, :], rb[:, 0:512], MULT)
                nc.vector.tensor_tensor(attnT[64:128, hp, ts(qb, 512)],
                                        ob[0:64, :], rb[:, 512:1024], MULT)

        if DEBUG_DUMP:
            nc.sync.dma_start(dbg_qP[:], qP[:])
            nc.sync.dma_start(dbg_kP[:], kP[:])
            nc.sync.dma_start(dbg_va[:], v_aug[:])
            nc.sync.dma_start(dbg_at[:], attnT[:])
        # tail: out-projection of the last query block
        for it in outproj_items(QB - 1):
            it()

        stk.close()

    nc.finalize()
    return nc


def kernel(q, k, v, mask, w_q, b_q, w_k, b_k, w_v, b_v, w_o, b_o):
    global _NC, LAST_EXEC_NS, LAST_TRACE
    if _NC is None:
        _NC = _build()
    nc = _NC

    q = np.asarray(q, np.float32)
    k = np.asarray(k, np.float32)
    v = np.asarray(v, np.float32)
    w_q = np.asarray(w_q, np.float32)
    w_k = np.asarray(w_k, np.float32)
    w_v = np.asarray(w_v, np.float32)
    w_o = np.asarray(w_o, np.float32)
    b_q = np.asarray(b_q, np.float32)
    b_k = np.asarray(b_k, np.float32)
    b_v = np.asarray(b_v, np.float32)
    b_o = np.asarray(b_o, np.float32)

    in_maps = []
    for c in range(8):
        b, hf = divmod(c, 2)
        sl = slice(hf * HALF, (hf + 1) * HALF)
        in_maps.append({
            "qT": q[b].T.astype(BF),
            "kT": k[b].T.astype(BF),
            "vT": v[b].T.astype(BF),
            "wq": w_q[sl, :].T.astype(BF),
            "wk": w_k[sl, :].T.astype(BF),
            "wv": w_v[sl, :].T.astype(BF),
            "wo": w_o[:, sl].T.astype(BF),
            "bq": b_q[sl].reshape(1, HALF).astype(BF),
            "bk": b_k[sl].reshape(1, HALF).astype(BF),
            "bv": b_v[sl].reshape(1, HALF).astype(BF),
        })

    kwargs = {}
    if TRACE:
        kwargs = dict(trace=True, trace_cores=[0])
    try:
        res = run_bass_kernel_spmd(nc, in_maps, core_ids=list(range(8)), **kwargs)
    except Exception:
        # transient device wedge (e.g. a previously killed client left a core
        # dirty) usually clears on retry
        time.sleep(2.0)
        res = run_bass_kernel_spmd(nc, in_maps, core_ids=list(range(8)), **kwargs)
    if TRACE:
        LAST_EXEC_NS = res.exec_time_ns
        LAST_TRACE = res.instructions_and_trace[1] if res.instructions_and_trace else None

    out = np.empty((B, S, D), np.float32)
    for b in range(B):
        out[b] = res.results[2 * b]["out"] + res.results[2 * b + 1]["out"] + b_o[None, :]
    return out
